# revision 32
# baseline (speedup 1.0000x reference)
"""Trainium2 Bass kernel for nn_LocallyConnected3 (B=128, C_in=32, C_out=8, S=8192).

  h[b,j,s]   = tanh(x[b,j,s] * sum_i w1[i,j,s])
  out[b,o,s] = tanh(sum_j h[b,j,s] * w2[o,j,s] + bias[o,s])

Sharding: S axis split across 8 cores (1024 positions each).

Per-core layout: SBUF partitions carry (s4, j) with s4 in 0..3 (position
sub-block) and j in 0..31 (in-channel); free dims carry (b, s_in).  All
device-side data is fp16 (PSUM accumulates fp32): x/w1/w2/bias/out move
over HBM at 2 bytes/elem.  Stage 2 is a packed matmul: for each s_in,
lhsT = h[(s4,j), b] (stationary), rhs = block-diag w2 [(s4,j), (o,s4)]
built host-side, so one matmul contracts j for 4 positions at once
with k=128.  DMA issue is split across queues by issuing engine
(x/w1/bias on qSync, w2/out on the GpSimd SWDGE queue) to overlap
transfers.  PSUM
[b, (s_in, o, s4)] accumulates 16 s_in per bank; bias enters via a
k=1 ones-matmul opener.  Output is stored tile-packed and unscrambled
on the host.
"""
import sys

sys.path.insert(0, '/opt/trn_rl_repo')

import numpy as np
import ml_dtypes

import concourse.bass as bass
import concourse.tile as tile
from concourse import mybir
from concourse.bass_utils import run_bass_kernel_spmd

N_CORES = 8
B = 128          # batch
CJ = 32          # C_in
CO = 8           # C_out
S = 8192
SC = S // N_CORES   # 1024 positions per core
SIN = 32            # s_in per s-tile
ST = 4 * SIN        # 128 positions per s-tile (4 s4-blocks x SIN s_in)
NT = SC // ST       # 8 s-tiles per core
GSZ = 16            # s_in per psum accumulation group (one 2KB bank)
NG = SIN // GSZ     # 2 psum groups per s-tile
NQ = 2              # b-chunks of tile 0 (pipeline-fill granularity)
BQ = B // NQ        # 64
F32 = mybir.dt.float32
F16 = mybir.dt.float16
NPF16 = np.float16


def _patch_tile_drain():
    """core_v3 CTRL instructions accept a single sync-wait; stock
    TileContext packs every final sem wait onto one InstDrain and the pinned
    neuronxcc rejects it.  Spread the waits over single-wait nops."""
    from concourse.tile import ScopedClock, TileContext

    if getattr(TileContext, '_drain_patched', False):
        return

    def _drain_and_barrier_split(self, tick_clock, wait_clock):
        nc = self.nc
        drain_inst = nc.sync.drain()
        wait_clock.add_sem_waits(
            drain_inst.ins, ScopedClock({None: tick_clock.global_clock})
        )
        si = drain_inst.ins.sync_info
        if si is not None and si.on_wait and len(si.on_wait) > 1:
            waits = list(si.on_wait)
            si.on_wait = waits[:1]
            for w in waits[1:]:
                nop = nc.sync.nop(nofuse=True, hint="drain_wait_split")
                nsi = nop.ins.sync_info
                if nsi is None:
                    import bass_rust
                    nop.ins.sync_info = bass_rust.SyncInfo(on_wait=[w], on_update=[])
                else:
                    nsi.on_wait = [w]
        nc.all_engine_barrier()
        assert self.sems is not None
        popped = nc._tile_sem_poison_stack.pop()
        assert popped is self._sem_poison
        nc.clear_and_free_semaphores(list(self.sems.allocated().values()))
        nc.all_engine_barrier()

    TileContext._drain_and_barrier = _drain_and_barrier_split
    TileContext._drain_patched = True


def _build_nc():
    nc = bass.Bass("TRN2")
    # tile 0 (pipeline fill): w1 + x in b-chunks, p=(s4,j)
    w10_d = nc.declare_dram_parameter("w10", [128, CJ, SIN], F16, isOutput=False)
    x0_d = nc.declare_dram_parameter("x0q", [NQ, 128, BQ, SIN], F16, isOutput=False)
    # tiles 1..NT-1: one combined [w1 | x] block per tile (single DMA each
    # keeps the qSync queue's per-DMA overhead off the critical path)
    xw_d = nc.declare_dram_parameter(
        "xw", [NT - 1, 128, CJ + B, SIN], F16, isOutput=False
    )
    # padded block-diag w2: [t, k=(s4,j), s_in, (o, s4')]
    w2b_d = nc.declare_dram_parameter("w2b", [NT, 128, SIN, CO, 4], F16, isOutput=False)
    # bias: [(t,g,k)=NT*SIN, o, s4]
    bias_d = nc.declare_dram_parameter("biasb", [NT * SIN, CO, 4], F16, isOutput=False)
    # tile-packed output, host unscrambles: [t, b, s_in, o, s4]
    out_d = nc.declare_dram_parameter("out", [NT, B, SIN, CO, 4], F16, isOutput=True)

    with tile.TileContext(nc) as tc:
        with (
            tc.tile_pool(name="x0p", bufs=NQ) as x0p,
            tc.tile_pool(name="w10p", bufs=1) as w10p,
            tc.tile_pool(name="xwp", bufs=3) as xwp,
            tc.tile_pool(name="hp", bufs=3) as hp,
            tc.tile_pool(name="w2p", bufs=3) as w2p,
            tc.tile_pool(name="bp", bufs=1) as bp,
            tc.tile_pool(name="sp", bufs=3) as sp,
            tc.tile_pool(name="pp", bufs=4, space="PSUM") as pp,
        ):
            bias_t = bp.tile([1, NT * SIN, CO, 4], F16)
            ones_t = bp.tile([1, 128], F16)
            nc.vector.memset(ones_t[:], 1.0)

            # Software-pipelined: ACT is in-order with no lookahead, so
            # stage-2 tanh (which waits on PE) is emitted AFTER the next
            # tile's stage-1 tanh to avoid head-of-line blocking on ACT.
            pending = []  # (t, [ps tiles]) awaiting stage-2 ACT + store

            def drain_stage2():
                t, ps = pending.pop(0)
                st = sp.tile([128, SIN, CO, 4], F16, name="st")
                nc.scalar.activation(
                    st[:], ps[:], mybir.ActivationFunctionType.Tanh
                )
                if t == NT - 1:
                    # tail: the last store goes on the idle HWDGE sync queue
                    nc.sync.dma_start(out_d[t], st[:])
                else:
                    nc.gpsimd.dma_start(out_d[t], st[:])

            for t in range(NT):
                ht = hp.tile([128, B, SIN], F16)
                if t == 0:
                    # fill path: separate w1 + chunked x, tanh per chunk so
                    # the ACT pipe starts as early as possible
                    w1t = w10p.tile([128, CJ, SIN], F16)
                    nc.sync.dma_start(w1t[:], w10_d[:])
                    xq = []
                    for q in range(NQ):
                        xc = x0p.tile([128, BQ, SIN], F16, name="xc")
                        nc.sync.dma_start(xc[:], x0_d[q])
                        xq.append(xc)
                    nc.sync.dma_start(bias_t[0:1], bias_d[:].unsqueeze(0))
                    for step in (16, 8, 4, 2, 1):
                        nc.vector.tensor_add(
                            w1t[:, 0:step, :], w1t[:, 0:step, :],
                            w1t[:, step:2 * step, :],
                        )
                    for q in range(NQ):
                        hsl = ht[:, q * BQ:(q + 1) * BQ, :]
                        nc.vector.tensor_mul(
                            hsl, xq[q][:],
                            w1t[:, 0:1, :].broadcast_to([128, BQ, SIN]),
                        )
                        nc.scalar.activation(
                            hsl, hsl, mybir.ActivationFunctionType.Tanh
                        )
                else:
                    # steady state: one combined [w1 | x] DMA per tile
                    xwt = xwp.tile([128, CJ + B, SIN], F16)
                    nc.sync.dma_start(xwt[:], xw_d[t - 1])
                    for step in (16, 8, 4, 2, 1):
                        nc.vector.tensor_add(
                            xwt[:, 0:step, :], xwt[:, 0:step, :],
                            xwt[:, step:2 * step, :],
                        )
                    nc.vector.tensor_mul(
                        ht[:], xwt[:, CJ:CJ + B, :],
                        xwt[:, 0:1, :].broadcast_to([128, B, SIN]),
                    )
                    nc.scalar.activation(
                        ht[:], ht[:], mybir.ActivationFunctionType.Tanh
                    )
                w2t = w2p.tile([128, SIN, CO, 4], F16)
                nc.gpsimd.dma_start(w2t[:], w2b_d[t])
                # ---- stage 2: packed matmuls; one 2-bank psum tile per
                # s-tile, accumulation groups of 16 s_in per bank ----
                ps = pp.tile([128, SIN, CO, 4], F32, name="ps")
                for g in range(NG):
                    nc.tensor.matmul(
                        ps[:, g * GSZ:(g + 1) * GSZ],
                        ones_t[:],
                        bias_t[0:1, (t * NG + g) * GSZ:(t * NG + g + 1) * GSZ],
                        start=True, stop=False,
                        skip_group_check=True,
                    )
                    for k in range(GSZ):
                        si = g * GSZ + k
                        nc.tensor.matmul(
                            ps[:, si],
                            ht[:, :, si],        # lhsT [(s4,j), b]
                            w2t[:, si],          # rhs  [(s4,j), (o,s4)]
                            start=False, stop=(k == GSZ - 1),
                            skip_group_check=True,
                        )
                pending.append((t, ps))
                if len(pending) > 2:
                    drain_stage2()
            while pending:
                drain_stage2()
    _split_multi_waits(nc)
    return nc


def _split_multi_waits(nc):
    """core_v3 CTRL sync accepts one wait per instruction (2 for EventSem).
    Hoist excess waits onto same-engine nofuse nops inserted just before."""
    for fn in nc.m.functions:
        for blk in fn.blocks:
            insts = list(blk.instructions)
            if not any(
                i.sync_info is not None and i.sync_info.on_wait
                and len(i.sync_info.on_wait) > 1
                for i in insts
            ):
                continue
            new = []
            for inst in insts:
                si = inst.sync_info
                cap = 2 if isinstance(inst, mybir.InstEventSemaphore) else 1
                if si is not None and si.on_wait and len(si.on_wait) > cap:
                    waits = list(si.on_wait)
                    si.on_wait = waits[:cap]
                    for k, w in enumerate(waits[cap:]):
                        new.append(mybir.InstNoOp(
                            name=f"{inst.name}-ws{k}",
                            engine=inst.engine,
                            bass_nofuse=True,
                            sync_info=mybir.SyncInfo(on_wait=[w], on_update=[]),
                        ))
                new.append(inst)
            try:
                blk.instructions = new
            except AttributeError:
                blk.instructions[:] = new


def _pack_inputs(x, w1, w2, bias):
    """Shard on S and build the per-core packed side tensors (fp16)."""
    in_maps = []
    for c in range(N_CORES):
        sl = slice(c * SC, (c + 1) * SC)
        # x packed two ways: [t, p, b, si] for the combined stream and
        # [q, p, b_chunk, si] chunks for tile 0's fill path
        xall = (
            x[:, :, sl].reshape(B, CJ, NT, 4, SIN)
            .transpose(2, 3, 1, 0, 4).reshape(NT, 128, B, SIN)
        ).astype(NPF16)
        x0q = np.ascontiguousarray(
            xall[0].reshape(128, NQ, BQ, SIN).transpose(1, 0, 2, 3)
        )
        w1c = (
            w1[:, :, sl].reshape(CJ, CJ, NT, 4, SIN)
            .transpose(2, 3, 1, 0, 4).reshape(NT, 128, CJ, SIN)
        ).astype(NPF16)
        w10 = np.ascontiguousarray(w1c[0])
        # combined [w1 | x] per tile for tiles 1..NT-1
        xw = np.ascontiguousarray(
            np.concatenate([w1c[1:], xall[1:]], axis=2)
        )
        # padded block-diag w2: [t, (s4,j), s_in, (o, s4')], nonzero at s4'==s4
        w2r = w2[:, :, sl].reshape(CO, CJ, NT, 4, SIN)      # o j t s4 si
        M = np.zeros((NT, 128, SIN, CO, 4), np.float16)
        for s4 in range(4):
            M[:, s4 * 32:(s4 + 1) * 32, :, :, s4] = (
                w2r[:, :, :, s4, :].transpose(2, 1, 3, 0)   # t j si o
            )
        w2b = M
        # bias: [(t,g,k)=(t,s_in), o, s4] with s = t*ST + s4*SIN + s_in
        biasb = np.ascontiguousarray(
            bias[:, sl].reshape(CO, NT, 4, SIN)
            .transpose(1, 3, 0, 2).reshape(NT * SIN, CO, 4)
        ).astype(NPF16)
        in_maps.append({
            "w10": w10, "x0q": x0q, "xw": xw, "w2b": w2b, "biasb": biasb,
        })
    return in_maps


_CACHED_NC = None


def kernel(x, w1, w2, bias):
    global _CACHED_NC
    _patch_tile_drain()
    x = np.asarray(x, np.float32)
    w1 = np.asarray(w1, np.float32)
    w2 = np.asarray(w2, np.float32)
    bias = np.asarray(bias, np.float32)

    if _CACHED_NC is None:
        _CACHED_NC = _build_nc()
    nc = _CACHED_NC

    in_maps = _pack_inputs(x, w1, w2, bias)
    res = run_bass_kernel_spmd(nc, in_maps, list(range(N_CORES)))
    # out_d[t, b, s_in, o, s4] -> out[b, o, c*SC + t*ST + s4*SIN + s_in]
    outs = []
    for c in range(N_CORES):
        oc = np.asarray(res.results[c]["out"]).astype(np.float32)
        outs.append(oc.transpose(1, 3, 0, 4, 2).reshape(B, CO, SC))
    return np.concatenate(outs, axis=2)


if __name__ == "__main__":
    rng = np.random.default_rng(0)
    x = rng.standard_normal((B, CJ, S), dtype=np.float32)
    w1 = rng.standard_normal((CJ, CJ, S), dtype=np.float32)
    w2 = rng.standard_normal((CO, CJ, S), dtype=np.float32)
    bias = rng.standard_normal((CO, S), dtype=np.float32)
    out = kernel(x=x, w1=w1, w2=w2, bias=bias)
    h = np.tanh(x * w1.sum(0, keepdims=True))
    ref = np.tanh(np.einsum('bjs,ojs->bos', h, w2) + bias[None])
    err = np.abs(out - ref).max() / max(np.abs(ref).max(), 1e-9)
    rel = np.linalg.norm(out - ref) / np.linalg.norm(ref)
    print("self-check max err:", err, "rel:", rel)


# revision 33
# speedup vs baseline: 1.0527x; 1.0527x over previous
"""Trainium2 Bass kernel for nn_LocallyConnected3 (B=128, C_in=32, C_out=8, S=8192).

  h[b,j,s]   = tanh(x[b,j,s] * sum_i w1[i,j,s])
  out[b,o,s] = tanh(sum_j h[b,j,s] * w2[o,j,s] + bias[o,s])

Sharding: S axis split across 8 cores (1024 positions each).

Per-core layout: SBUF partitions carry (s4, j) with s4 in 0..3 (position
sub-block) and j in 0..31 (in-channel); free dims carry (b, s_in).  All
device-side data is fp16 (PSUM accumulates fp32): x/w1/w2/bias/out move
over HBM at 2 bytes/elem.  Stage 2 is a packed matmul: for each s_in,
lhsT = h[(s4,j), b] (stationary), rhs = block-diag w2 [(s4,j), (o,s4)]
built host-side, so one matmul contracts j for 4 positions at once
with k=128.  DMA issue is split across queues by issuing engine
(x/w1/bias on qSync, w2/out on the GpSimd SWDGE queue) to overlap
transfers.  PSUM
[b, (s_in, o, s4)] accumulates 16 s_in per bank; bias enters via a
k=1 ones-matmul opener.  Output is stored tile-packed and unscrambled
on the host.
"""
import sys

sys.path.insert(0, '/opt/trn_rl_repo')

import numpy as np
import ml_dtypes

import concourse.bass as bass
import concourse.tile as tile
from concourse import mybir
from concourse.bass_utils import run_bass_kernel_spmd

N_CORES = 8
B = 128          # batch
CJ = 32          # C_in
CO = 8           # C_out
S = 8192
SC = S // N_CORES   # 1024 positions per core
SIN = 32            # s_in per s-tile
ST = 4 * SIN        # 128 positions per s-tile (4 s4-blocks x SIN s_in)
NT = SC // ST       # 8 s-tiles per core
GSZ = 16            # s_in per psum accumulation group (one 2KB bank)
NG = SIN // GSZ     # 2 psum groups per s-tile
NQ = 2              # b-chunks of tile 0 (pipeline-fill granularity)
BQ = B // NQ        # 64
F32 = mybir.dt.float32
F16 = mybir.dt.float16
NPF16 = np.float16


def _patch_tile_drain():
    """core_v3 CTRL instructions accept a single sync-wait; stock
    TileContext packs every final sem wait onto one InstDrain and the pinned
    neuronxcc rejects it.  Spread the waits over single-wait nops."""
    from concourse.tile import ScopedClock, TileContext

    if getattr(TileContext, '_drain_patched', False):
        return

    def _drain_and_barrier_split(self, tick_clock, wait_clock):
        nc = self.nc
        drain_inst = nc.sync.drain()
        wait_clock.add_sem_waits(
            drain_inst.ins, ScopedClock({None: tick_clock.global_clock})
        )
        si = drain_inst.ins.sync_info
        if si is not None and si.on_wait and len(si.on_wait) > 1:
            waits = list(si.on_wait)
            si.on_wait = waits[:1]
            for w in waits[1:]:
                nop = nc.sync.nop(nofuse=True, hint="drain_wait_split")
                nsi = nop.ins.sync_info
                if nsi is None:
                    import bass_rust
                    nop.ins.sync_info = bass_rust.SyncInfo(on_wait=[w], on_update=[])
                else:
                    nsi.on_wait = [w]
        nc.all_engine_barrier()
        assert self.sems is not None
        popped = nc._tile_sem_poison_stack.pop()
        assert popped is self._sem_poison
        nc.clear_and_free_semaphores(list(self.sems.allocated().values()))
        nc.all_engine_barrier()

    TileContext._drain_and_barrier = _drain_and_barrier_split
    TileContext._drain_patched = True


def _build_nc():
    nc = bass.Bass("TRN2")
    # tile 0 (pipeline fill): w1 + x in b-chunks, p=(s4,j)
    w10_d = nc.declare_dram_parameter("w10", [128, CJ, SIN], F16, isOutput=False)
    x0_d = nc.declare_dram_parameter("x0q", [NQ, 128, BQ, SIN], F16, isOutput=False)
    # tiles 1..NT-1: one combined [w1 | x] block per tile (single DMA each
    # keeps the qSync queue's per-DMA overhead off the critical path)
    xw_d = nc.declare_dram_parameter(
        "xw", [NT - 1, 128, CJ + B, SIN], F16, isOutput=False
    )
    # padded block-diag w2: [t, k=(s4,j), s_in, (o, s4')]
    w2b_d = nc.declare_dram_parameter("w2b", [NT, 128, SIN, CO, 4], F16, isOutput=False)
    # bias: [(t,g,k)=NT*SIN, o, s4]
    bias_d = nc.declare_dram_parameter("biasb", [NT * SIN, CO, 4], F16, isOutput=False)
    # tile-packed output, host unscrambles: [t, b, s_in, o, s4]
    out_d = nc.declare_dram_parameter("out", [NT, B, SIN, CO, 4], F16, isOutput=True)

    with tile.TileContext(nc) as tc:
        with (
            tc.tile_pool(name="x0p", bufs=NQ) as x0p,
            tc.tile_pool(name="w10p", bufs=1) as w10p,
            tc.tile_pool(name="xwp", bufs=4) as xwp,
            tc.tile_pool(name="hp", bufs=4) as hp,
            tc.tile_pool(name="w2p", bufs=3) as w2p,
            tc.tile_pool(name="bp", bufs=1) as bp,
            tc.tile_pool(name="sp", bufs=3) as sp,
            tc.tile_pool(name="pp", bufs=4, space="PSUM") as pp,
        ):
            bias_t = bp.tile([1, NT * SIN, CO, 4], F16)
            ones_t = bp.tile([1, 128], F16)
            nc.vector.memset(ones_t[:], 1.0)

            # Software-pipelined: ACT is in-order with no lookahead, so
            # stage-2 tanh (which waits on PE) is emitted AFTER the next
            # tile's stage-1 tanh to avoid head-of-line blocking on ACT.
            pending = []  # (t, [ps tiles]) awaiting stage-2 ACT + store

            def drain_stage2():
                t, ps = pending.pop(0)
                st = sp.tile([128, SIN, CO, 4], F16, name="st")
                nc.scalar.activation(
                    st[:], ps[:], mybir.ActivationFunctionType.Tanh
                )
                if t == NT - 1:
                    # tail: the last store goes on the idle HWDGE sync queue
                    nc.sync.dma_start(out_d[t], st[:])
                else:
                    nc.gpsimd.dma_start(out_d[t], st[:])

            for t in range(NT):
                ht = hp.tile([128, B, SIN], F16)
                if t == 0:
                    # fill path: separate w1 + chunked x, tanh per chunk so
                    # the ACT pipe starts as early as possible
                    w1t = w10p.tile([128, CJ, SIN], F16)
                    nc.sync.dma_start(w1t[:], w10_d[:])
                    xq = []
                    for q in range(NQ):
                        xc = x0p.tile([128, BQ, SIN], F16, name="xc")
                        nc.sync.dma_start(xc[:], x0_d[q])
                        xq.append(xc)
                    nc.sync.dma_start(bias_t[0:1], bias_d[:].unsqueeze(0))
                    for step in (16, 8, 4, 2, 1):
                        nc.vector.tensor_add(
                            w1t[:, 0:step, :], w1t[:, 0:step, :],
                            w1t[:, step:2 * step, :],
                        )
                    for q in range(NQ):
                        hsl = ht[:, q * BQ:(q + 1) * BQ, :]
                        nc.vector.tensor_mul(
                            hsl, xq[q][:],
                            w1t[:, 0:1, :].broadcast_to([128, BQ, SIN]),
                        )
                        nc.scalar.activation(
                            hsl, hsl, mybir.ActivationFunctionType.Tanh
                        )
                else:
                    # steady state: combined [w1 | x] tile, loaded as two
                    # slice-DMAs so the multiply can start on the first half
                    xwt = xwp.tile([128, CJ + B, SIN], F16)
                    HB = B // 2
                    nc.sync.dma_start(
                        xwt[:, 0:CJ + HB, :], xw_d[t - 1, :, 0:CJ + HB, :]
                    )
                    nc.sync.dma_start(
                        xwt[:, CJ + HB:, :], xw_d[t - 1, :, CJ + HB:, :]
                    )
                    for step in (16, 8, 4, 2, 1):
                        nc.vector.tensor_add(
                            xwt[:, 0:step, :], xwt[:, 0:step, :],
                            xwt[:, step:2 * step, :],
                        )
                    nc.vector.tensor_mul(
                        ht[:, 0:HB, :], xwt[:, CJ:CJ + HB, :],
                        xwt[:, 0:1, :].broadcast_to([128, HB, SIN]),
                    )
                    nc.vector.tensor_mul(
                        ht[:, HB:, :], xwt[:, CJ + HB:, :],
                        xwt[:, 0:1, :].broadcast_to([128, B - HB, SIN]),
                    )
                    nc.scalar.activation(
                        ht[:], ht[:], mybir.ActivationFunctionType.Tanh
                    )
                w2t = w2p.tile([128, SIN, CO, 4], F16)
                nc.gpsimd.dma_start(w2t[:], w2b_d[t])
                # ---- stage 2: packed matmuls; one 2-bank psum tile per
                # s-tile, accumulation groups of 16 s_in per bank ----
                ps = pp.tile([128, SIN, CO, 4], F32, name="ps")
                for g in range(NG):
                    nc.tensor.matmul(
                        ps[:, g * GSZ:(g + 1) * GSZ],
                        ones_t[:],
                        bias_t[0:1, (t * NG + g) * GSZ:(t * NG + g + 1) * GSZ],
                        start=True, stop=False,
                        skip_group_check=True,
                    )
                    for k in range(GSZ):
                        si = g * GSZ + k
                        nc.tensor.matmul(
                            ps[:, si],
                            ht[:, :, si],        # lhsT [(s4,j), b]
                            w2t[:, si],          # rhs  [(s4,j), (o,s4)]
                            start=False, stop=(k == GSZ - 1),
                            skip_group_check=True,
                        )
                pending.append((t, ps))
                if len(pending) > 2:
                    drain_stage2()
            while pending:
                drain_stage2()
    _split_multi_waits(nc)
    return nc


def _split_multi_waits(nc):
    """core_v3 CTRL sync accepts one wait per instruction (2 for EventSem).
    Hoist excess waits onto same-engine nofuse nops inserted just before."""
    for fn in nc.m.functions:
        for blk in fn.blocks:
            insts = list(blk.instructions)
            if not any(
                i.sync_info is not None and i.sync_info.on_wait
                and len(i.sync_info.on_wait) > 1
                for i in insts
            ):
                continue
            new = []
            for inst in insts:
                si = inst.sync_info
                cap = 2 if isinstance(inst, mybir.InstEventSemaphore) else 1
                if si is not None and si.on_wait and len(si.on_wait) > cap:
                    waits = list(si.on_wait)
                    si.on_wait = waits[:cap]
                    for k, w in enumerate(waits[cap:]):
                        new.append(mybir.InstNoOp(
                            name=f"{inst.name}-ws{k}",
                            engine=inst.engine,
                            bass_nofuse=True,
                            sync_info=mybir.SyncInfo(on_wait=[w], on_update=[]),
                        ))
                new.append(inst)
            try:
                blk.instructions = new
            except AttributeError:
                blk.instructions[:] = new


def _pack_inputs(x, w1, w2, bias):
    """Shard on S and build the per-core packed side tensors (fp16)."""
    in_maps = []
    for c in range(N_CORES):
        sl = slice(c * SC, (c + 1) * SC)
        # x packed two ways: [t, p, b, si] for the combined stream and
        # [q, p, b_chunk, si] chunks for tile 0's fill path
        xall = (
            x[:, :, sl].reshape(B, CJ, NT, 4, SIN)
            .transpose(2, 3, 1, 0, 4).reshape(NT, 128, B, SIN)
        ).astype(NPF16)
        x0q = np.ascontiguousarray(
            xall[0].reshape(128, NQ, BQ, SIN).transpose(1, 0, 2, 3)
        )
        w1c = (
            w1[:, :, sl].reshape(CJ, CJ, NT, 4, SIN)
            .transpose(2, 3, 1, 0, 4).reshape(NT, 128, CJ, SIN)
        ).astype(NPF16)
        w10 = np.ascontiguousarray(w1c[0])
        # combined [w1 | x] per tile for tiles 1..NT-1
        xw = np.ascontiguousarray(
            np.concatenate([w1c[1:], xall[1:]], axis=2)
        )
        # padded block-diag w2: [t, (s4,j), s_in, (o, s4')], nonzero at s4'==s4
        w2r = w2[:, :, sl].reshape(CO, CJ, NT, 4, SIN)      # o j t s4 si
        M = np.zeros((NT, 128, SIN, CO, 4), np.float16)
        for s4 in range(4):
            M[:, s4 * 32:(s4 + 1) * 32, :, :, s4] = (
                w2r[:, :, :, s4, :].transpose(2, 1, 3, 0)   # t j si o
            )
        w2b = M
        # bias: [(t,g,k)=(t,s_in), o, s4] with s = t*ST + s4*SIN + s_in
        biasb = np.ascontiguousarray(
            bias[:, sl].reshape(CO, NT, 4, SIN)
            .transpose(1, 3, 0, 2).reshape(NT * SIN, CO, 4)
        ).astype(NPF16)
        in_maps.append({
            "w10": w10, "x0q": x0q, "xw": xw, "w2b": w2b, "biasb": biasb,
        })
    return in_maps


_CACHED_NC = None


def kernel(x, w1, w2, bias):
    global _CACHED_NC
    _patch_tile_drain()
    x = np.asarray(x, np.float32)
    w1 = np.asarray(w1, np.float32)
    w2 = np.asarray(w2, np.float32)
    bias = np.asarray(bias, np.float32)

    if _CACHED_NC is None:
        _CACHED_NC = _build_nc()
    nc = _CACHED_NC

    in_maps = _pack_inputs(x, w1, w2, bias)
    res = run_bass_kernel_spmd(nc, in_maps, list(range(N_CORES)))
    # out_d[t, b, s_in, o, s4] -> out[b, o, c*SC + t*ST + s4*SIN + s_in]
    outs = []
    for c in range(N_CORES):
        oc = np.asarray(res.results[c]["out"]).astype(np.float32)
        outs.append(oc.transpose(1, 3, 0, 4, 2).reshape(B, CO, SC))
    return np.concatenate(outs, axis=2)


if __name__ == "__main__":
    rng = np.random.default_rng(0)
    x = rng.standard_normal((B, CJ, S), dtype=np.float32)
    w1 = rng.standard_normal((CJ, CJ, S), dtype=np.float32)
    w2 = rng.standard_normal((CO, CJ, S), dtype=np.float32)
    bias = rng.standard_normal((CO, S), dtype=np.float32)
    out = kernel(x=x, w1=w1, w2=w2, bias=bias)
    h = np.tanh(x * w1.sum(0, keepdims=True))
    ref = np.tanh(np.einsum('bjs,ojs->bos', h, w2) + bias[None])
    err = np.abs(out - ref).max() / max(np.abs(ref).max(), 1e-9)
    rel = np.linalg.norm(out - ref) / np.linalg.norm(ref)
    print("self-check max err:", err, "rel:", rel)


# revision 34
# speedup vs baseline: 1.0710x; 1.0174x over previous
"""Trainium2 Bass kernel for nn_LocallyConnected3 (B=128, C_in=32, C_out=8, S=8192).

  h[b,j,s]   = tanh(x[b,j,s] * sum_i w1[i,j,s])
  out[b,o,s] = tanh(sum_j h[b,j,s] * w2[o,j,s] + bias[o,s])

Sharding: S axis split across 8 cores (1024 positions each).

Per-core layout: SBUF partitions carry (s4, j) with s4 in 0..3 (position
sub-block) and j in 0..31 (in-channel); free dims carry (b, s_in).  All
device-side data is fp16 (PSUM accumulates fp32): x/w1/w2/bias/out move
over HBM at 2 bytes/elem.  Stage 2 is a packed matmul: for each s_in,
lhsT = h[(s4,j), b] (stationary), rhs = block-diag w2 [(s4,j), (o,s4)]
built host-side, so one matmul contracts j for 4 positions at once
with k=128.  DMA issue is split across queues by issuing engine
(x/w1/bias on qSync, w2/out on the GpSimd SWDGE queue) to overlap
transfers.  PSUM
[b, (s_in, o, s4)] accumulates 16 s_in per bank; bias enters via a
k=1 ones-matmul opener.  Output is stored tile-packed and unscrambled
on the host.
"""
import sys

sys.path.insert(0, '/opt/trn_rl_repo')

import numpy as np
import ml_dtypes

import concourse.bass as bass
import concourse.tile as tile
from concourse import mybir
from concourse.bass_utils import run_bass_kernel_spmd

N_CORES = 8
B = 128          # batch
CJ = 32          # C_in
CO = 8           # C_out
S = 8192
SC = S // N_CORES   # 1024 positions per core
SIN = 32            # s_in per s-tile
ST = 4 * SIN        # 128 positions per s-tile (4 s4-blocks x SIN s_in)
NT = SC // ST       # 8 s-tiles per core
GSZ = 16            # s_in per psum accumulation group (one 2KB bank)
NG = SIN // GSZ     # 2 psum groups per s-tile
NQ = 2              # b-chunks of tile 0 (pipeline-fill granularity)
BQ = B // NQ        # 64
F32 = mybir.dt.float32
F16 = mybir.dt.float16
NPF16 = np.float16


def _patch_tile_drain():
    """core_v3 CTRL instructions accept a single sync-wait; stock
    TileContext packs every final sem wait onto one InstDrain and the pinned
    neuronxcc rejects it.  Spread the waits over single-wait nops."""
    from concourse.tile import ScopedClock, TileContext

    if getattr(TileContext, '_drain_patched', False):
        return

    def _drain_and_barrier_split(self, tick_clock, wait_clock):
        nc = self.nc
        drain_inst = nc.sync.drain()
        wait_clock.add_sem_waits(
            drain_inst.ins, ScopedClock({None: tick_clock.global_clock})
        )
        si = drain_inst.ins.sync_info
        if si is not None and si.on_wait and len(si.on_wait) > 1:
            waits = list(si.on_wait)
            si.on_wait = waits[:1]
            for w in waits[1:]:
                nop = nc.sync.nop(nofuse=True, hint="drain_wait_split")
                nsi = nop.ins.sync_info
                if nsi is None:
                    import bass_rust
                    nop.ins.sync_info = bass_rust.SyncInfo(on_wait=[w], on_update=[])
                else:
                    nsi.on_wait = [w]
        nc.all_engine_barrier()
        assert self.sems is not None
        popped = nc._tile_sem_poison_stack.pop()
        assert popped is self._sem_poison
        nc.clear_and_free_semaphores(list(self.sems.allocated().values()))
        nc.all_engine_barrier()

    TileContext._drain_and_barrier = _drain_and_barrier_split
    TileContext._drain_patched = True


def _build_nc():
    nc = bass.Bass("TRN2")
    # tile 0 (pipeline fill): w1 + x in b-chunks, p=(s4,j)
    w10_d = nc.declare_dram_parameter("w10", [128, CJ, SIN], F16, isOutput=False)
    x0_d = nc.declare_dram_parameter("x0q", [NQ, 128, BQ, SIN], F16, isOutput=False)
    # tiles 1..NT-1: one combined [w1 | x] block per tile (single DMA each
    # keeps the qSync queue's per-DMA overhead off the critical path)
    xw_d = nc.declare_dram_parameter(
        "xw", [NT - 1, 128, CJ + B, SIN], F16, isOutput=False
    )
    # padded block-diag w2: [t, k=(s4,j), s_in, (o, s4')]
    w2b_d = nc.declare_dram_parameter("w2b", [NT, 128, SIN, CO, 4], F16, isOutput=False)
    # bias: [(t,g,k)=NT*SIN, o, s4]
    bias_d = nc.declare_dram_parameter("biasb", [NT * SIN, CO, 4], F16, isOutput=False)
    # tile-packed output, host unscrambles: [t, b, s_in, o, s4]
    out_d = nc.declare_dram_parameter("out", [NT, B, SIN, CO, 4], F16, isOutput=True)

    with tile.TileContext(nc) as tc:
        with (
            tc.tile_pool(name="x0p", bufs=NQ) as x0p,
            tc.tile_pool(name="w10p", bufs=1) as w10p,
            tc.tile_pool(name="xwp", bufs=4) as xwp,
            tc.tile_pool(name="hp", bufs=4) as hp,
            tc.tile_pool(name="w2p", bufs=3) as w2p,
            tc.tile_pool(name="bp", bufs=1) as bp,
            tc.tile_pool(name="sp", bufs=3) as sp,
            tc.tile_pool(name="pp", bufs=4, space="PSUM") as pp,
        ):
            bias_t = bp.tile([1, NT * SIN, CO, 4], F16)
            ones_t = bp.tile([1, 128], F16)
            nc.vector.memset(ones_t[:], 1.0)

            # Software-pipelined: ACT is in-order with no lookahead, so
            # stage-2 tanh (which waits on PE) is emitted AFTER the next
            # tile's stage-1 tanh to avoid head-of-line blocking on ACT.
            pending = []  # (t, [ps tiles]) awaiting stage-2 ACT + store

            def drain_stage2():
                t, ps = pending.pop(0)
                st = sp.tile([128, SIN, CO, 4], F16, name="st")
                nc.scalar.activation(
                    st[:], ps[:], mybir.ActivationFunctionType.Tanh
                )
                if t == NT - 1:
                    # tail: the last store goes on the idle HWDGE sync queue
                    nc.sync.dma_start(out_d[t], st[:])
                else:
                    nc.gpsimd.dma_start(out_d[t], st[:])

            for t in range(NT):
                ht = hp.tile([128, B, SIN], F16)
                if t == 0:
                    # fill path: separate w1 + chunked x, tanh per chunk so
                    # the ACT pipe starts as early as possible
                    w1t = w10p.tile([128, CJ, SIN], F16)
                    nc.sync.dma_start(w1t[:], w10_d[:])
                    # x0 chunks ride the gpsimd queue so qSync can start
                    # streaming xw(1) right behind w1; bias goes out on the
                    # (idle during preamble) scalar HWDGE queue
                    xq = []
                    for q in range(NQ):
                        xc = x0p.tile([128, BQ, SIN], F16, name="xc")
                        nc.gpsimd.dma_start(xc[:], x0_d[q])
                        xq.append(xc)
                    nc.scalar.dma_start(bias_t[0:1], bias_d[:].unsqueeze(0))
                    for step in (16, 8, 4, 2, 1):
                        nc.vector.tensor_add(
                            w1t[:, 0:step, :], w1t[:, 0:step, :],
                            w1t[:, step:2 * step, :],
                        )
                    for q in range(NQ):
                        hsl = ht[:, q * BQ:(q + 1) * BQ, :]
                        nc.vector.tensor_mul(
                            hsl, xq[q][:],
                            w1t[:, 0:1, :].broadcast_to([128, BQ, SIN]),
                        )
                        nc.scalar.activation(
                            hsl, hsl, mybir.ActivationFunctionType.Tanh
                        )
                else:
                    # steady state: combined [w1 | x] tile, loaded as two
                    # slice-DMAs so the multiply can start on the first half
                    xwt = xwp.tile([128, CJ + B, SIN], F16)
                    HB = B // 2
                    nc.sync.dma_start(
                        xwt[:, 0:CJ + HB, :], xw_d[t - 1, :, 0:CJ + HB, :]
                    )
                    nc.sync.dma_start(
                        xwt[:, CJ + HB:, :], xw_d[t - 1, :, CJ + HB:, :]
                    )
                    for step in (16, 8, 4, 2, 1):
                        nc.vector.tensor_add(
                            xwt[:, 0:step, :], xwt[:, 0:step, :],
                            xwt[:, step:2 * step, :],
                        )
                    nc.vector.tensor_mul(
                        ht[:, 0:HB, :], xwt[:, CJ:CJ + HB, :],
                        xwt[:, 0:1, :].broadcast_to([128, HB, SIN]),
                    )
                    nc.vector.tensor_mul(
                        ht[:, HB:, :], xwt[:, CJ + HB:, :],
                        xwt[:, 0:1, :].broadcast_to([128, B - HB, SIN]),
                    )
                    nc.scalar.activation(
                        ht[:], ht[:], mybir.ActivationFunctionType.Tanh
                    )
                w2t = w2p.tile([128, SIN, CO, 4], F16)
                nc.gpsimd.dma_start(w2t[:], w2b_d[t])
                # ---- stage 2: packed matmuls; one 2-bank psum tile per
                # s-tile, accumulation groups of 16 s_in per bank ----
                ps = pp.tile([128, SIN, CO, 4], F32, name="ps")
                for g in range(NG):
                    nc.tensor.matmul(
                        ps[:, g * GSZ:(g + 1) * GSZ],
                        ones_t[:],
                        bias_t[0:1, (t * NG + g) * GSZ:(t * NG + g + 1) * GSZ],
                        start=True, stop=False,
                        skip_group_check=True,
                    )
                    for k in range(GSZ):
                        si = g * GSZ + k
                        nc.tensor.matmul(
                            ps[:, si],
                            ht[:, :, si],        # lhsT [(s4,j), b]
                            w2t[:, si],          # rhs  [(s4,j), (o,s4)]
                            start=False, stop=(k == GSZ - 1),
                            skip_group_check=True,
                        )
                pending.append((t, ps))
                if len(pending) > 2:
                    drain_stage2()
            while pending:
                drain_stage2()
    _split_multi_waits(nc)
    return nc


def _split_multi_waits(nc):
    """core_v3 CTRL sync accepts one wait per instruction (2 for EventSem).
    Hoist excess waits onto same-engine nofuse nops inserted just before."""
    for fn in nc.m.functions:
        for blk in fn.blocks:
            insts = list(blk.instructions)
            if not any(
                i.sync_info is not None and i.sync_info.on_wait
                and len(i.sync_info.on_wait) > 1
                for i in insts
            ):
                continue
            new = []
            for inst in insts:
                si = inst.sync_info
                cap = 2 if isinstance(inst, mybir.InstEventSemaphore) else 1
                if si is not None and si.on_wait and len(si.on_wait) > cap:
                    waits = list(si.on_wait)
                    si.on_wait = waits[:cap]
                    for k, w in enumerate(waits[cap:]):
                        new.append(mybir.InstNoOp(
                            name=f"{inst.name}-ws{k}",
                            engine=inst.engine,
                            bass_nofuse=True,
                            sync_info=mybir.SyncInfo(on_wait=[w], on_update=[]),
                        ))
                new.append(inst)
            try:
                blk.instructions = new
            except AttributeError:
                blk.instructions[:] = new


def _pack_inputs(x, w1, w2, bias):
    """Shard on S and build the per-core packed side tensors (fp16)."""
    in_maps = []
    for c in range(N_CORES):
        sl = slice(c * SC, (c + 1) * SC)
        # x packed two ways: [t, p, b, si] for the combined stream and
        # [q, p, b_chunk, si] chunks for tile 0's fill path
        xall = (
            x[:, :, sl].reshape(B, CJ, NT, 4, SIN)
            .transpose(2, 3, 1, 0, 4).reshape(NT, 128, B, SIN)
        ).astype(NPF16)
        x0q = np.ascontiguousarray(
            xall[0].reshape(128, NQ, BQ, SIN).transpose(1, 0, 2, 3)
        )
        w1c = (
            w1[:, :, sl].reshape(CJ, CJ, NT, 4, SIN)
            .transpose(2, 3, 1, 0, 4).reshape(NT, 128, CJ, SIN)
        ).astype(NPF16)
        w10 = np.ascontiguousarray(w1c[0])
        # combined [w1 | x] per tile for tiles 1..NT-1
        xw = np.ascontiguousarray(
            np.concatenate([w1c[1:], xall[1:]], axis=2)
        )
        # padded block-diag w2: [t, (s4,j), s_in, (o, s4')], nonzero at s4'==s4
        w2r = w2[:, :, sl].reshape(CO, CJ, NT, 4, SIN)      # o j t s4 si
        M = np.zeros((NT, 128, SIN, CO, 4), np.float16)
        for s4 in range(4):
            M[:, s4 * 32:(s4 + 1) * 32, :, :, s4] = (
                w2r[:, :, :, s4, :].transpose(2, 1, 3, 0)   # t j si o
            )
        w2b = M
        # bias: [(t,g,k)=(t,s_in), o, s4] with s = t*ST + s4*SIN + s_in
        biasb = np.ascontiguousarray(
            bias[:, sl].reshape(CO, NT, 4, SIN)
            .transpose(1, 3, 0, 2).reshape(NT * SIN, CO, 4)
        ).astype(NPF16)
        in_maps.append({
            "w10": w10, "x0q": x0q, "xw": xw, "w2b": w2b, "biasb": biasb,
        })
    return in_maps


_CACHED_NC = None


def kernel(x, w1, w2, bias):
    global _CACHED_NC
    _patch_tile_drain()
    x = np.asarray(x, np.float32)
    w1 = np.asarray(w1, np.float32)
    w2 = np.asarray(w2, np.float32)
    bias = np.asarray(bias, np.float32)

    if _CACHED_NC is None:
        _CACHED_NC = _build_nc()
    nc = _CACHED_NC

    in_maps = _pack_inputs(x, w1, w2, bias)
    res = run_bass_kernel_spmd(nc, in_maps, list(range(N_CORES)))
    # out_d[t, b, s_in, o, s4] -> out[b, o, c*SC + t*ST + s4*SIN + s_in]
    outs = []
    for c in range(N_CORES):
        oc = np.asarray(res.results[c]["out"]).astype(np.float32)
        outs.append(oc.transpose(1, 3, 0, 4, 2).reshape(B, CO, SC))
    return np.concatenate(outs, axis=2)


if __name__ == "__main__":
    rng = np.random.default_rng(0)
    x = rng.standard_normal((B, CJ, S), dtype=np.float32)
    w1 = rng.standard_normal((CJ, CJ, S), dtype=np.float32)
    w2 = rng.standard_normal((CO, CJ, S), dtype=np.float32)
    bias = rng.standard_normal((CO, S), dtype=np.float32)
    out = kernel(x=x, w1=w1, w2=w2, bias=bias)
    h = np.tanh(x * w1.sum(0, keepdims=True))
    ref = np.tanh(np.einsum('bjs,ojs->bos', h, w2) + bias[None])
    err = np.abs(out - ref).max() / max(np.abs(ref).max(), 1e-9)
    rel = np.linalg.norm(out - ref) / np.linalg.norm(ref)
    print("self-check max err:", err, "rel:", rel)


# revision 35
# speedup vs baseline: 1.1250x; 1.0504x over previous
"""Trainium2 Bass kernel for nn_LocallyConnected3 (B=128, C_in=32, C_out=8, S=8192).

  h[b,j,s]   = tanh(x[b,j,s] * sum_i w1[i,j,s])
  out[b,o,s] = tanh(sum_j h[b,j,s] * w2[o,j,s] + bias[o,s])

Sharding: S axis split across 8 cores (1024 positions each).

Per-core layout: SBUF partitions carry (s4, j) with s4 in 0..3 (position
sub-block) and j in 0..31 (in-channel); free dims carry (b, s_in).  All
device-side data is fp16 (PSUM accumulates fp32): x/w1/w2/bias/out move
over HBM at 2 bytes/elem.  Stage 2 is a packed matmul: for each s_in,
lhsT = h[(s4,j), b] (stationary), rhs = block-diag w2 [(s4,j), (o,s4)]
built host-side, so one matmul contracts j for 4 positions at once
with k=128.  DMA issue is split across queues by issuing engine
(x/w1/bias on qSync, w2/out on the GpSimd SWDGE queue) to overlap
transfers.  PSUM
[b, (s_in, o, s4)] accumulates 16 s_in per bank; bias enters via a
k=1 ones-matmul opener.  Output is stored tile-packed and unscrambled
on the host.
"""
import sys

sys.path.insert(0, '/opt/trn_rl_repo')

import numpy as np
import ml_dtypes

import concourse.bass as bass
import concourse.tile as tile
from concourse import mybir
from concourse.bass_utils import run_bass_kernel_spmd

N_CORES = 8
B = 128          # batch
CJ = 32          # C_in
CO = 8           # C_out
S = 8192
SC = S // N_CORES   # 1024 positions per core
SIN = 32            # s_in per s-tile
ST = 4 * SIN        # 128 positions per s-tile (4 s4-blocks x SIN s_in)
NT = SC // ST       # 8 s-tiles per core
GSZ = 16            # s_in per psum accumulation group (one 2KB bank)
NG = SIN // GSZ     # 2 psum groups per s-tile
NQ = 2              # b-chunks of tile 0 (pipeline-fill granularity)
BQ = B // NQ        # 64
F32 = mybir.dt.float32
F16 = mybir.dt.float16
NPF16 = np.float16


def _patch_tile_drain():
    """core_v3 CTRL instructions accept a single sync-wait; stock
    TileContext packs every final sem wait onto one InstDrain and the pinned
    neuronxcc rejects it.  Spread the waits over single-wait nops."""
    from concourse.tile import ScopedClock, TileContext

    if getattr(TileContext, '_drain_patched', False):
        return

    def _drain_and_barrier_split(self, tick_clock, wait_clock):
        nc = self.nc
        drain_inst = nc.sync.drain()
        wait_clock.add_sem_waits(
            drain_inst.ins, ScopedClock({None: tick_clock.global_clock})
        )
        si = drain_inst.ins.sync_info
        if si is not None and si.on_wait and len(si.on_wait) > 1:
            waits = list(si.on_wait)
            si.on_wait = waits[:1]
            for w in waits[1:]:
                nop = nc.sync.nop(nofuse=True, hint="drain_wait_split")
                nsi = nop.ins.sync_info
                if nsi is None:
                    import bass_rust
                    nop.ins.sync_info = bass_rust.SyncInfo(on_wait=[w], on_update=[])
                else:
                    nsi.on_wait = [w]
        nc.all_engine_barrier()
        assert self.sems is not None
        popped = nc._tile_sem_poison_stack.pop()
        assert popped is self._sem_poison
        nc.clear_and_free_semaphores(list(self.sems.allocated().values()))
        nc.all_engine_barrier()

    TileContext._drain_and_barrier = _drain_and_barrier_split
    TileContext._drain_patched = True


def _build_nc():
    nc = bass.Bass("TRN2")
    # tile 0 (pipeline fill): w1 + x in b-chunks, p=(s4,j)
    w10_d = nc.declare_dram_parameter("w10", [128, CJ, SIN], F16, isOutput=False)
    x0_d = nc.declare_dram_parameter("x0q", [NQ, 128, BQ, SIN], F16, isOutput=False)
    # tiles 1..NT-1: one combined [w1 | x] block per tile (single DMA each
    # keeps the qSync queue's per-DMA overhead off the critical path)
    xw_d = nc.declare_dram_parameter(
        "xw", [NT - 1, 128, CJ + B, SIN], F16, isOutput=False
    )
    # padded block-diag w2: [t, k=(s4,j), s_in, (o, s4')]
    w2b_d = nc.declare_dram_parameter("w2b", [NT, 128, SIN, CO, 4], F16, isOutput=False)
    # bias: [(t,g,k)=NT*SIN, o, s4]
    bias_d = nc.declare_dram_parameter("biasb", [NT * SIN, CO, 4], F16, isOutput=False)
    # tile-packed output, host unscrambles: [t, b, s_in, o, s4]
    out_d = nc.declare_dram_parameter("out", [NT, B, SIN, CO, 4], F16, isOutput=True)

    with tile.TileContext(nc) as tc:
        with (
            tc.tile_pool(name="x0p", bufs=NQ) as x0p,
            tc.tile_pool(name="w10p", bufs=1) as w10p,
            tc.tile_pool(name="xwp", bufs=4) as xwp,
            tc.tile_pool(name="hp", bufs=4) as hp,
            tc.tile_pool(name="w2p", bufs=3) as w2p,
            tc.tile_pool(name="bp", bufs=1) as bp,
            tc.tile_pool(name="sp", bufs=3) as sp,
            tc.tile_pool(name="pp", bufs=4, space="PSUM") as pp,
        ):
            bias_t = bp.tile([1, NT * SIN, CO, 4], F16)
            ones_t = bp.tile([1, 128], F16)
            nc.vector.memset(ones_t[:], 1.0)

            # Software-pipelined: ACT is in-order with no lookahead, so
            # stage-2 tanh (which waits on PE) is emitted AFTER the next
            # tile's stage-1 tanh to avoid head-of-line blocking on ACT.
            pending = []  # (t, [ps tiles]) awaiting stage-2 ACT + store

            def drain_stage2():
                t, ps = pending.pop(0)
                st = sp.tile([128, SIN, CO, 4], F16, name="st")
                if t == NT - 1:
                    # tail: split tanh+store in halves so the first store
                    # overlaps the second tanh; use the idle HWDGE sync queue
                    for g in range(NG):
                        gs = slice(g * GSZ, (g + 1) * GSZ)
                        nc.scalar.activation(
                            st[:, gs], ps[:, gs],
                            mybir.ActivationFunctionType.Tanh,
                        )
                        nc.sync.dma_start(out_d[t, :, gs], st[:, gs])
                else:
                    nc.scalar.activation(
                        st[:], ps[:], mybir.ActivationFunctionType.Tanh
                    )
                    nc.gpsimd.dma_start(out_d[t], st[:])

            for t in range(NT):
                ht = hp.tile([128, B, SIN], F16)
                if t == 0:
                    # fill path: separate w1 + chunked x, tanh per chunk so
                    # the ACT pipe starts as early as possible
                    w1t = w10p.tile([128, CJ, SIN], F16)
                    nc.sync.dma_start(w1t[:], w10_d[:])
                    # x0 chunks ride the gpsimd queue so qSync can start
                    # streaming xw(1) right behind w1; bias goes out on the
                    # (idle during preamble) scalar HWDGE queue
                    xq = []
                    for q in range(NQ):
                        xc = x0p.tile([128, BQ, SIN], F16, name="xc")
                        nc.gpsimd.dma_start(xc[:], x0_d[q])
                        xq.append(xc)
                    nc.scalar.dma_start(bias_t[0:1], bias_d[:].unsqueeze(0))
                    for step in (16, 8, 4, 2, 1):
                        nc.vector.tensor_add(
                            w1t[:, 0:step, :], w1t[:, 0:step, :],
                            w1t[:, step:2 * step, :],
                        )
                    for q in range(NQ):
                        hsl = ht[:, q * BQ:(q + 1) * BQ, :]
                        nc.vector.tensor_mul(
                            hsl, xq[q][:],
                            w1t[:, 0:1, :].broadcast_to([128, BQ, SIN]),
                        )
                        nc.scalar.activation(
                            hsl, hsl, mybir.ActivationFunctionType.Tanh
                        )
                else:
                    # steady state: combined [w1 | x] tile, loaded as two
                    # slice-DMAs so the multiply can start on the first half
                    xwt = xwp.tile([128, CJ + B, SIN], F16)
                    HB = B // 2
                    nc.sync.dma_start(
                        xwt[:, 0:CJ + HB, :], xw_d[t - 1, :, 0:CJ + HB, :]
                    )
                    nc.sync.dma_start(
                        xwt[:, CJ + HB:, :], xw_d[t - 1, :, CJ + HB:, :]
                    )
                    for step in (16, 8, 4, 2, 1):
                        nc.vector.tensor_add(
                            xwt[:, 0:step, :], xwt[:, 0:step, :],
                            xwt[:, step:2 * step, :],
                        )
                    nc.vector.tensor_mul(
                        ht[:, 0:HB, :], xwt[:, CJ:CJ + HB, :],
                        xwt[:, 0:1, :].broadcast_to([128, HB, SIN]),
                    )
                    if t <= 2:
                        # ramp: tanh the first half as soon as it's ready
                        nc.scalar.activation(
                            ht[:, 0:HB, :], ht[:, 0:HB, :],
                            mybir.ActivationFunctionType.Tanh,
                        )
                    nc.vector.tensor_mul(
                        ht[:, HB:, :], xwt[:, CJ + HB:, :],
                        xwt[:, 0:1, :].broadcast_to([128, B - HB, SIN]),
                    )
                    if t <= 2:
                        nc.scalar.activation(
                            ht[:, HB:, :], ht[:, HB:, :],
                            mybir.ActivationFunctionType.Tanh,
                        )
                    else:
                        nc.scalar.activation(
                            ht[:], ht[:], mybir.ActivationFunctionType.Tanh
                        )
                w2t = w2p.tile([128, SIN, CO, 4], F16)
                nc.gpsimd.dma_start(w2t[:], w2b_d[t])
                # ---- stage 2: packed matmuls; one 2-bank psum tile per
                # s-tile, accumulation groups of 16 s_in per bank ----
                ps = pp.tile([128, SIN, CO, 4], F32, name="ps")
                for g in range(NG):
                    nc.tensor.matmul(
                        ps[:, g * GSZ:(g + 1) * GSZ],
                        ones_t[:],
                        bias_t[0:1, (t * NG + g) * GSZ:(t * NG + g + 1) * GSZ],
                        start=True, stop=False,
                        skip_group_check=True,
                    )
                    for k in range(GSZ):
                        si = g * GSZ + k
                        nc.tensor.matmul(
                            ps[:, si],
                            ht[:, :, si],        # lhsT [(s4,j), b]
                            w2t[:, si],          # rhs  [(s4,j), (o,s4)]
                            start=False, stop=(k == GSZ - 1),
                            skip_group_check=True,
                        )
                pending.append((t, ps))
                if len(pending) > 2:
                    drain_stage2()
            while pending:
                drain_stage2()
    _split_multi_waits(nc)
    return nc


def _split_multi_waits(nc):
    """core_v3 CTRL sync accepts one wait per instruction (2 for EventSem).
    Hoist excess waits onto same-engine nofuse nops inserted just before."""
    for fn in nc.m.functions:
        for blk in fn.blocks:
            insts = list(blk.instructions)
            if not any(
                i.sync_info is not None and i.sync_info.on_wait
                and len(i.sync_info.on_wait) > 1
                for i in insts
            ):
                continue
            new = []
            for inst in insts:
                si = inst.sync_info
                cap = 2 if isinstance(inst, mybir.InstEventSemaphore) else 1
                if si is not None and si.on_wait and len(si.on_wait) > cap:
                    waits = list(si.on_wait)
                    si.on_wait = waits[:cap]
                    for k, w in enumerate(waits[cap:]):
                        new.append(mybir.InstNoOp(
                            name=f"{inst.name}-ws{k}",
                            engine=inst.engine,
                            bass_nofuse=True,
                            sync_info=mybir.SyncInfo(on_wait=[w], on_update=[]),
                        ))
                new.append(inst)
            try:
                blk.instructions = new
            except AttributeError:
                blk.instructions[:] = new


def _pack_inputs(x, w1, w2, bias):
    """Shard on S and build the per-core packed side tensors (fp16)."""
    in_maps = []
    for c in range(N_CORES):
        sl = slice(c * SC, (c + 1) * SC)
        # x packed two ways: [t, p, b, si] for the combined stream and
        # [q, p, b_chunk, si] chunks for tile 0's fill path
        xall = (
            x[:, :, sl].reshape(B, CJ, NT, 4, SIN)
            .transpose(2, 3, 1, 0, 4).reshape(NT, 128, B, SIN)
        ).astype(NPF16)
        x0q = np.ascontiguousarray(
            xall[0].reshape(128, NQ, BQ, SIN).transpose(1, 0, 2, 3)
        )
        w1c = (
            w1[:, :, sl].reshape(CJ, CJ, NT, 4, SIN)
            .transpose(2, 3, 1, 0, 4).reshape(NT, 128, CJ, SIN)
        ).astype(NPF16)
        w10 = np.ascontiguousarray(w1c[0])
        # combined [w1 | x] per tile for tiles 1..NT-1
        xw = np.ascontiguousarray(
            np.concatenate([w1c[1:], xall[1:]], axis=2)
        )
        # padded block-diag w2: [t, (s4,j), s_in, (o, s4')], nonzero at s4'==s4
        w2r = w2[:, :, sl].reshape(CO, CJ, NT, 4, SIN)      # o j t s4 si
        M = np.zeros((NT, 128, SIN, CO, 4), np.float16)
        for s4 in range(4):
            M[:, s4 * 32:(s4 + 1) * 32, :, :, s4] = (
                w2r[:, :, :, s4, :].transpose(2, 1, 3, 0)   # t j si o
            )
        w2b = M
        # bias: [(t,g,k)=(t,s_in), o, s4] with s = t*ST + s4*SIN + s_in
        biasb = np.ascontiguousarray(
            bias[:, sl].reshape(CO, NT, 4, SIN)
            .transpose(1, 3, 0, 2).reshape(NT * SIN, CO, 4)
        ).astype(NPF16)
        in_maps.append({
            "w10": w10, "x0q": x0q, "xw": xw, "w2b": w2b, "biasb": biasb,
        })
    return in_maps


_CACHED_NC = None


def kernel(x, w1, w2, bias):
    global _CACHED_NC
    _patch_tile_drain()
    x = np.asarray(x, np.float32)
    w1 = np.asarray(w1, np.float32)
    w2 = np.asarray(w2, np.float32)
    bias = np.asarray(bias, np.float32)

    if _CACHED_NC is None:
        _CACHED_NC = _build_nc()
    nc = _CACHED_NC

    in_maps = _pack_inputs(x, w1, w2, bias)
    res = run_bass_kernel_spmd(nc, in_maps, list(range(N_CORES)))
    # out_d[t, b, s_in, o, s4] -> out[b, o, c*SC + t*ST + s4*SIN + s_in]
    outs = []
    for c in range(N_CORES):
        oc = np.asarray(res.results[c]["out"]).astype(np.float32)
        outs.append(oc.transpose(1, 3, 0, 4, 2).reshape(B, CO, SC))
    return np.concatenate(outs, axis=2)


if __name__ == "__main__":
    rng = np.random.default_rng(0)
    x = rng.standard_normal((B, CJ, S), dtype=np.float32)
    w1 = rng.standard_normal((CJ, CJ, S), dtype=np.float32)
    w2 = rng.standard_normal((CO, CJ, S), dtype=np.float32)
    bias = rng.standard_normal((CO, S), dtype=np.float32)
    out = kernel(x=x, w1=w1, w2=w2, bias=bias)
    h = np.tanh(x * w1.sum(0, keepdims=True))
    ref = np.tanh(np.einsum('bjs,ojs->bos', h, w2) + bias[None])
    err = np.abs(out - ref).max() / max(np.abs(ref).max(), 1e-9)
    rel = np.linalg.norm(out - ref) / np.linalg.norm(ref)
    print("self-check max err:", err, "rel:", rel)
